# revision 3
# baseline (speedup 1.0000x reference)
"""Trainium2 Bass kernel v2: causal MHA with extra time-mix matrix D.

attn = D @ softmax(causal(Q K^T / 8)) @ V per head, concat heads, out-proj.
Shapes: B=4, T=2048, d=1024, H=16, e=64. Sharding: batch (4) x head-group (2).

Design vs baseline:
- bf16 for x/W projections, Q/K/V, probabilities, PV partials, and D (error
  ~5e-3 vs 2e-2 tolerance); f32 psum accumulation; out-projection f32r.
- Transposed PV: stationary = exp(scores) chunk [k,q], moving = V [k,65].
  bf16 has no minimum moving-width, so this costs 65 rows per 128q x 128k
  instead of 128 — half the fp32r-layout cost — and the PV partial lands
  directly in [q, e] orientation: no PE transposes, normalization is one
  reciprocal + one tensor_scalar per 128q block.
- Causal diagonal blocks refined to 128-wide query windows; the mask is one
  128x128 lower-tri tile applied to the leading 128 columns of each window.
- Software-pipelined chunk loop: PV of chunk c-1 is emitted after scores of
  chunk c (engine queues are strictly in-order; this hides the exp latency).
- The D @ PV contraction is decomposed into per-query-block partials that
  accumulate in SBUF and are drained one item per chunk inside the attention
  loop, filling what PE slack remains; out-projection per qb interleaves with
  the last partials.
- bias rows removed from projections (DVE adds bias on the psum->sbuf copy).
- x / dT prefetched ahead on the SP queue; stores go on the ACT DGE queue.
"""

import sys

for _p in ("/opt/trn_rl_repo", "/root/.axon_site/_ro/trn_rl_repo"):
    if _p not in sys.path:
        sys.path.append(_p)

from contextlib import ExitStack

import numpy as np

import concourse.bass as bass  # noqa: F401
import concourse.tile as tile
from concourse import bacc, mybir
from concourse.bass_utils import run_bass_kernel_spmd

dt = mybir.dt

B, T, D, H, E = 4, 2048, 1024, 16, 64
EV = E + 2       # vt per-head stride (pad to even: bf16 offsets 4B-aligned)
HG = 8
COEF = 1.0 / E ** 0.5
P = 128
TQB = 512
NTQ = T // TQB   # 4
NTC = T // P     # 16
ND = D // P      # 8

_CACHED_NC = None


def _build_nc():
    import os
    _phase = os.environ.get("KPHASE", "all")
    nc = bacc.Bacc("TRN2", target_bir_lowering=False, debug=False)
    f32, f32r, bf16 = dt.float32, dt.float32r, dt.bfloat16
    Exp = mybir.ActivationFunctionType.Exp
    mult = mybir.AluOpType.mult
    add = mybir.AluOpType.add

    xqT = nc.dram_tensor("xqT", [D, T], bf16, kind="ExternalInput").ap()
    xkT = nc.dram_tensor("xkT", [D, T], bf16, kind="ExternalInput").ap()
    xvT = nc.dram_tensor("xvT", [D, T], bf16, kind="ExternalInput").ap()
    wqT = nc.dram_tensor("wqT", [D, 512], bf16, kind="ExternalInput").ap()
    wkT = nc.dram_tensor("wkT", [D, 512], bf16, kind="ExternalInput").ap()
    wvT = nc.dram_tensor("wvT", [D, 512], bf16, kind="ExternalInput").ap()
    bqc = nc.dram_tensor("bqc", [P, NTQ], f32, kind="ExternalInput").ap()
    bkc = nc.dram_tensor("bkc", [P, NTQ], f32, kind="ExternalInput").ap()
    bvr = nc.dram_tensor("bvr", [P, 512], f32, kind="ExternalInput").ap()
    woT = nc.dram_tensor("woT", [512, D], f32r, kind="ExternalInput").ap()
    dTt = nc.dram_tensor("dT", [T, T], bf16, kind="ExternalInput").ap()
    tri = nc.dram_tensor("tri", [P, P], bf16, kind="ExternalInput").ap()
    y = nc.dram_tensor("y", [T, D], f32, kind="ExternalOutput").ap()

    with tile.TileContext(nc) as tc, ExitStack() as ctx:
        # ---- persistent sbuf pools ----------------------------------------
        consts = ctx.enter_context(tc.tile_pool(name="consts", bufs=1))
        wpool = ctx.enter_context(tc.tile_pool(name="w", bufs=1))
        kvp = ctx.enter_context(tc.tile_pool(name="kv", bufs=1))
        qtp = ctx.enter_context(tc.tile_pool(name="qts", bufs=2))
        pvgp = ctx.enter_context(tc.tile_pool(name="pvg", bufs=1))
        xsp = ctx.enter_context(tc.tile_pool(name="xs", bufs=1))
        dtp = ctx.enter_context(tc.tile_pool(name="dtp", bufs=28))
        a2p = ctx.enter_context(tc.tile_pool(name="a2acc", bufs=1))

        def load_xb(xdram, tb, tag):
            xb = xsp.tile([P, ND * 512], bf16, tag=tag)
            v = xb[:].rearrange("p (c w) -> p c w", c=ND)
            s = xdram.rearrange("(c p) t -> p c t", p=P)
            for hcl in range(2):
                nc.sync.dma_start(
                    v[:, 4 * hcl:4 * (hcl + 1), :],
                    s[:, 4 * hcl:4 * (hcl + 1), TQB * tb:TQB * (tb + 1)])
            return xb

        # load order on the SP queue = first-use order
        wv = wpool.tile([P, ND * 512], bf16, tag="wv")
        wvv = wv[:].rearrange("p (c w) -> p c w", c=ND)
        wvs = wvT.rearrange("(c p) w -> p c w", p=P)
        nc.sync.dma_start(wvv[:, 0:1, :], wvs[:, 0:1, :])
        xb_cur = {"v": load_xb(xvT, 0, "xv")}
        nc.sync.dma_start(wvv[:, 1:ND, :], wvs[:, 1:ND, :])
        bvt = consts.tile([P, 512], f32, tag="bvt")
        nc.sync.dma_start(bvt[:], bvr[:])
        wq = wpool.tile([P, ND * 512], bf16, tag="wq")
        nc.sync.dma_start(
            wq[:].rearrange("p (c w) -> p c w", c=ND),
            wqT.rearrange("(c p) w -> p c w", p=P))
        xb_cur["q"] = load_xb(xqT, 0, "xq")
        wk = wpool.tile([P, ND * 512], bf16, tag="wk")
        nc.sync.dma_start(
            wk[:].rearrange("p (c w) -> p c w", c=ND),
            wkT.rearrange("(c p) w -> p c w", p=P))
        xb_cur["k"] = load_xb(xkT, 0, "xk")

        bqt = consts.tile([P, NTQ], f32, tag="bqt")
        nc.sync.dma_start(bqt[:], bqc[:])
        bkt = consts.tile([P, NTQ], f32, tag="bkt")
        nc.sync.dma_start(bkt[:], bkc[:])
        trit = consts.tile([P, P], bf16, tag="trit")
        nc.sync.dma_start(trit[:], tri[:])

        ones32 = consts.tile([P, HG], f32, tag="ones32")
        nc.vector.memset(ones32[:], 1.0)
        onesb = consts.tile([P, HG], bf16, tag="onesb")
        nc.vector.tensor_copy(onesb[:], ones32[:])

        kt = [kvp.tile([P, T], bf16, tag=f"kt{p}", name=f"kt{p}") for p in range(NTQ)]
        vt = [kvp.tile([P, HG * EV], bf16, tag=f"vt{t}", name=f"vt{t}")
              for t in range(NTC)]
        for t in range(NTC):
            ones_dst = vt[t][:].rearrange("p (h c) -> p h c", c=EV)[:, :, E]
            nc.vector.tensor_copy(ones_dst, onesb[:])
        pvg = [pvgp.tile([P, 512], bf16, tag=f"pvg{t}", name=f"pvg{t}")
               for t in range(NTC)]
        a2acc = [[a2p.tile([P, 512], f32r, tag=f"a2_{qb}_{p}", name=f"a2_{qb}_{p}")
                  for p in range(4)] for qb in range(NTQ)]

        dts = []   # dT tile prefetch, i-major: n = 16*i + 4*qb + tl

        def issue_dt(cap):
            while len(dts) < min(cap, NTQ * NTC):
                n = len(dts)
                i_, r = n // 16, n % 16
                qb_, tl = r // 4, r % 4
                t_ = 4 * i_ + tl
                dtt = dtp.tile([P, 512], bf16, tag="dt")
                nc.sync.dma_start(
                    dtt[:],
                    dTt[P * t_:P * (t_ + 1), TQB * qb_:TQB * (qb_ + 1)])
                dts.append(dtt)

        # ---- interleaved projections + attention + D@PV partials ----------
        with tc.tile_pool(name="mp", bufs=2, space="PSUM") as mp, \
             tc.tile_pool(name="sps", bufs=2, space="PSUM") as sps, \
             tc.tile_pool(name="pvps", bufs=2, space="PSUM") as pvps, \
             tc.tile_pool(name="u", bufs=5) as upool, \
             tc.tile_pool(name="np", bufs=2) as npool:

            pa_st = {"tile": None, "spec": None, "tl": 0}

            def drain_pa(items):
                """Half of one D@PV partial (2 of 4 accumulating matmuls) per
                call — keeps per-chunk PE work balanced against the exp."""
                if pa_st["tile"] is None:
                    if not items:
                        return
                    pa_st["spec"] = items.pop(0)
                    pa_st["tile"] = mp.tile([P, 512], f32, tag="ps", name="pa")
                    pa_st["tl"] = 0
                i_, qb, p = pa_st["spec"]
                pa = pa_st["tile"]
                for tl in (pa_st["tl"], pa_st["tl"] + 1):
                    t = 4 * i_ + tl
                    nc.tensor.matmul(
                        pa[:], pvg[t][:, P * p:P * (p + 1)],
                        dts[16 * i_ + 4 * qb + tl][:],
                        start=(tl == 0), stop=(tl == 3))
                pa_st["tl"] += 2
                if pa_st["tl"] == 4:
                    if i_ == 0:
                        nc.vector.tensor_copy(a2acc[qb][p][:], pa[:])
                    else:
                        nc.vector.tensor_tensor(
                            a2acc[qb][p][:], a2acc[qb][p][:], pa[:], op=add)
                    pa_st["tile"] = None

            pa_items = []
            for tb in range(NTQ):
                # V projection for t-chunks 4tb..4tb+3
                xbv = xb_cur["v"]
                for tc_ in range(4):
                    ps = mp.tile([P, 512], f32, tag="ps")
                    for d in range(ND):
                        nc.tensor.matmul(
                            ps[:],
                            xbv[:, 512 * d + P * tc_:512 * d + P * (tc_ + 1)],
                            wv[:, 512 * d:512 * (d + 1)],
                            start=(d == 0), stop=(d == ND - 1))
                    t = 4 * tb + tc_
                    dst = vt[t][:].rearrange("p (h c) -> p h c", c=EV)[:, :, 0:E]
                    nc.vector.tensor_tensor(
                        dst, ps[:].rearrange("p (h c) -> p h c", c=E),
                        bvt[:].rearrange("p (h c) -> p h c", c=E), op=add)

                if _phase == "v":
                    continue

                # Q/K projections for this t-block
                qts = []
                xbq = xb_cur["q"]
                for p in range(NTQ):
                    psq = mp.tile([P, 512], f32, tag="ps")
                    for d in range(ND):
                        nc.tensor.matmul(
                            psq[:],
                            wq[:, 512 * d + P * p:512 * d + P * (p + 1)],
                            xbq[:, 512 * d:512 * (d + 1)],
                            start=(d == 0), stop=(d == ND - 1))
                    qt = qtp.tile([P, TQB], bf16, tag=f"qt{p}")
                    nc.vector.tensor_scalar(
                        qt[:], psq[:], bqt[:, p:p + 1], None, op0=add)
                    qts.append(qt)
                xbk = xb_cur["k"]
                for p in range(NTQ):
                    psk = mp.tile([P, 512], f32, tag="ps")
                    for d in range(ND):
                        nc.tensor.matmul(
                            psk[:],
                            wk[:, 512 * d + P * p:512 * d + P * (p + 1)],
                            xbk[:, 512 * d:512 * (d + 1)],
                            start=(d == 0), stop=(d == ND - 1))
                    nc.vector.tensor_scalar(
                        kt[p][:, TQB * tb:TQB * (tb + 1)], psk[:],
                        bkt[:, p:p + 1], None, op0=add)

                # prefetch next t-block's x while attention runs
                if tb + 1 < NTQ:
                    xb_cur = {"v": load_xb(xvT, tb + 1, "xv"),
                              "q": load_xb(xqT, tb + 1, "xq"),
                              "k": load_xb(xkT, tb + 1, "xk")}

                if _phase == "proj":
                    continue

                # ---- attention for query block i = tb ----------------------
                i = tb
                if _phase == "all":
                    issue_dt(16 * (i + 1))
                # (chunk, q-offset, width, masked): diag chunks get shrinking
                # 128-aligned windows; leading 128 cols hit the tri mask.
                chunks = [(c, 0, 512, False) for c in range(4 * i)]
                chunks += [(4 * i + j, 128 * j, 512 - 128 * j, True)
                           for j in range(4)]
                nch = len(chunks)
                # cap half-drains per i so backlog survives into the later,
                # slack-heavy query blocks (each full item = 2 half-drains)
                pa_cap = {0: 8, 1: 20, 2: 40, 3: 99}[i]
                pa_drained = 0
                for p in range(NTQ):
                    # PV partial accumulators in [q, e] orientation:
                    # pvq[0] = q-blocks 0,1; pvq[1] = q-blocks 2,3.
                    # Within a tile: [130*qsl + 65*h : +65], col 64 = rowsum.
                    pvq = [pvps.tile([P, 260], f32, tag=f"pvq{k}", name=f"pvq{k}")
                           for k in range(2)]
                    prev = None   # (chunk spec, u tile) pending PV emission
                    started = set()   # start=True zeroes the WHOLE 2KB psum
                    # bank (ZERO_REGION), so emit it exactly once per tile;
                    # later slices accumulate onto pending-zero bytes.

                    def emit_pv(spec, ut):
                        c, off, w, _m = spec
                        for qs in range(off // P, (off + w) // P):
                            k, qsl = qs // 2, qs % 2
                            for h in range(2):
                                first = k not in started
                                started.add(k)
                                nc.tensor.matmul(
                                    pvq[k][:, 130 * qsl + 65 * h:
                                           130 * qsl + 65 * h + 65],
                                    ut[:, w * h + P * qs - off:
                                       w * h + P * qs - off + P],
                                    vt[c][:, EV * (2 * p + h):
                                          EV * (2 * p + h) + E + 1],
                                    start=first,
                                    stop=(c == 4 * i + qs),
                                    skip_group_check=True)

                    for ci, (c, off, w, masked) in enumerate(chunks):
                        sph = []
                        for h in range(2):
                            sp = sps.tile([P, 512], f32, tag="sp")
                            nc.tensor.matmul(
                                sp[:, 0:w],
                                kt[p][64 * h:64 * (h + 1), P * c:P * (c + 1)],
                                qts[p][64 * h:64 * (h + 1), off:off + w],
                                start=True, stop=True)
                            sph.append(sp)
                        u = upool.tile([P, 1024], bf16, tag="u")
                        for h in range(2):
                            nc.scalar.activation(u[:, w * h:w * h + w],
                                                 sph[h][:, 0:w], Exp, scale=COEF)
                            if masked:
                                nc.vector.tensor_tensor(
                                    u[:, w * h:w * h + P],
                                    u[:, w * h:w * h + P],
                                    trit[:], op=mult)
                        if _phase == "all" and pa_drained < pa_cap:
                            drain_pa(pa_items)
                            pa_drained += 1
                        if prev is not None:
                            emit_pv(*prev)
                        prev = ((c, off, w, masked), u)
                    emit_pv(*prev)
                    # normalize into pvg ([t, head-col] layout, bf16)
                    rcp = npool.tile([P, 8], f32, tag="rcp")
                    for qs in range(4):
                        k, qsl = qs // 2, qs % 2
                        for h in range(2):
                            j = 2 * qs + h
                            nc.vector.reciprocal(
                                rcp[:, j:j + 1],
                                pvq[k][:, 130 * qsl + 65 * h + 64:
                                       130 * qsl + 65 * h + 65])
                            nc.vector.tensor_scalar(
                                pvg[4 * i + qs][:, E * (2 * p + h):
                                                E * (2 * p + h) + E],
                                pvq[k][:, 130 * qsl + 65 * h:
                                       130 * qsl + 65 * h + 64],
                                rcp[:, j:j + 1], None, op0=mult)
                    if _phase == "all":
                        pa_items += [(i, qb, p) for qb in range(NTQ)]
                        if i == 3 and p == 3:
                            wo = wpool.tile([P, 4 * D], f32r, tag="wo")
                            nc.sync.dma_start(
                                wo[:].rearrange("p (c w) -> p c w", c=4),
                                woT.rearrange("(c p) w -> p c w", p=P))

            # finish any half-emitted D@PV partial before psum pools close
            if _phase == "all" and pa_st["tile"] is not None:
                drain_pa(pa_items)

        # ---- drain remaining D@PV partials + out projection ---------------
        if _phase == "all":
            with tc.tile_pool(name="obuf", bufs=3) as obp, \
                 tc.tile_pool(name="mp2", bufs=2, space="PSUM") as mp2, \
                 tc.tile_pool(name="ops", bufs=4, space="PSUM") as ops:

                def drain_pa2(items):
                    if not items:
                        return
                    i_, qb, p = items.pop(0)
                    pa = mp2.tile([P, 512], f32, tag="ps")
                    for tl in range(4):
                        t = 4 * i_ + tl
                        nc.tensor.matmul(
                            pa[:], pvg[t][:, P * p:P * (p + 1)],
                            dts[16 * i_ + 4 * qb + tl][:],
                            start=(tl == 0), stop=(tl == 3))
                    nc.vector.tensor_tensor(
                        a2acc[qb][p][:], a2acc[qb][p][:], pa[:], op=add)

                byqb = [[it for it in pa_items if it[1] == q]
                        for q in range(NTQ + 1)]
                while byqb[0]:
                    drain_pa2(byqb[0])
                for qb in range(NTQ):
                    nxt = byqb[qb + 1]
                    for qs in range(4):
                        for nh in range(2):
                            drain_pa2(nxt)   # overlap next qb's partials
                            op_ = ops.tile([P, 512], f32, tag="op")
                            for cc in range(4):
                                nc.tensor.matmul(
                                    op_[:],
                                    a2acc[qb][cc][:, P * qs:P * (qs + 1)],
                                    wo[:, D * cc + 512 * nh:D * cc + 512 * (nh + 1)],
                                    start=(cc == 0), stop=(cc == 3))
                            ob = obp.tile([P, 512], f32, tag="ob")
                            nc.vector.tensor_copy(ob[:], op_[:])
                            nc.scalar.dma_start(
                                y[TQB * qb + P * qs:TQB * qb + P * (qs + 1),
                                  512 * nh:512 * (nh + 1)],
                                ob[:])
                    while nxt:
                        drain_pa2(nxt)

    nc.compile()
    return nc


def _prep_inputs(query_1, key_1, value_1, Wq, bq, Wk, bk, Wv, bv, Wo, bo, Dmat):
    """Host-side sharding: per-core input dicts."""
    import ml_dtypes
    f = np.float32
    bf = ml_dtypes.bfloat16

    def xT(x, b):
        return np.ascontiguousarray(np.asarray(x[b], f).T.astype(bf))

    wqTs, wkTs, wvTs, woTs, bqcs, bkcs, bvrs = [], [], [], [], [], [], []
    for g in range(2):
        h0 = HG * g
        wq = np.zeros((D, 512), f)
        wk = np.zeros((D, 512), f)
        bqcol = np.zeros((P, NTQ), f)
        bkcol = np.zeros((P, NTQ), f)
        for p in range(4):
            for h in range(2):
                hh = h0 + 2 * p + h
                c0 = 128 * p + 64 * h
                wq[:, c0:c0 + 64] = np.asarray(Wq[hh], f).T
                wk[:, c0:c0 + 64] = np.asarray(Wk[hh], f).T
                bqcol[64 * h:64 * (h + 1), p] = np.asarray(bq[hh], f)
                bkcol[64 * h:64 * (h + 1), p] = np.asarray(bk[hh], f)
        wv = np.zeros((D, 512), f)
        bvrow = np.zeros((512,), f)
        for j in range(HG):
            wv[:, 64 * j:64 * (j + 1)] = np.asarray(Wv[h0 + j], f).T
            bvrow[64 * j:64 * (j + 1)] = np.asarray(bv[h0 + j], f)
        wo = np.ascontiguousarray(np.asarray(Wo, f)[:, 64 * h0:64 * (h0 + HG)].T)
        wqTs.append(np.ascontiguousarray(wq.astype(bf)))
        wkTs.append(np.ascontiguousarray(wk.astype(bf)))
        wvTs.append(np.ascontiguousarray(wv.astype(bf)))
        woTs.append(wo)
        bqcs.append(bqcol)
        bkcs.append(bkcol)
        bvrs.append(np.ascontiguousarray(np.tile(bvrow[None, :], (P, 1))))

    dT = np.ascontiguousarray(np.asarray(Dmat, f).T.astype(bf))
    kl = np.arange(P)[:, None]
    ql = np.arange(P)[None, :]
    tric = np.ascontiguousarray((ql >= kl).astype(bf))

    xqTs = [xT(query_1, b) for b in range(B)]
    xkTs = [xT(key_1, b) for b in range(B)]
    xvTs = [xT(value_1, b) for b in range(B)]

    in_maps = []
    for c in range(8):
        b, g = c // 2, c % 2
        in_maps.append({
            "xqT": xqTs[b], "xkT": xkTs[b], "xvT": xvTs[b],
            "wqT": wqTs[g], "wkT": wkTs[g], "wvT": wvTs[g], "woT": woTs[g],
            "bqc": bqcs[g], "bkc": bkcs[g], "bvr": bvrs[g],
            "dT": dT, "tri": tric,
        })
    return in_maps


def kernel(query_1, key_1, value_1, Wq, bq, Wk, bk, Wv, bv, Wo, bo, D):
    import os
    os.environ["BASS_NEVER_TRACE"] = "1"  # NTFF capture hangs over the axon relay
    global _CACHED_NC
    if _CACHED_NC is None:
        _CACHED_NC = _build_nc()
    nc = _CACHED_NC
    in_maps = _prep_inputs(query_1, key_1, value_1, Wq, bq, Wk, bk, Wv, bv, Wo, bo, D)
    res = run_bass_kernel_spmd(nc, in_maps, core_ids=list(range(8)))
    bo_f = np.asarray(bo, np.float32)
    out = np.empty((B, T, 1024), np.float32)
    for b in range(B):
        out[b] = res.results[2 * b]["y"] + res.results[2 * b + 1]["y"] + bo_f
    return out
